# revision 5
# baseline (speedup 1.0000x reference)
"""Distance-encoded-bias multi-head self-attention on 8 Trainium2 NeuronCores.

Strategy
--------
Shard (batch b in 0..1) x (head-pair in 0..3) -> 8 cores. Each core computes
its 2 heads' full attention for its batch, plus the output-projection partial
for its heads' rows of proj_w; the host sums the 4 partials per batch.

Key algebraic moves (all exact):
 * Tokens are sorted by coordinate on the host (attention is permutation
   equivariant; output rows are inverse-permuted back).
 * cos(w|xi-xj|) = C_i C_j + S_i S_j with C=cos(w x), S=sin(w x), and
   sin(w|xi-xj|) = sign(xi-xj)(S_i C_j - C_i S_j). With sorted coords the
   sign is +1/-1 by block position (triangular only on diagonal blocks), so
   the whole Fourier bias becomes rank-2F matmuls -- no N^2 transcendentals.
 * The Gaussian local term E = exp(-d^2/ell^2) is precomputed on the host
   (head-independent: all heads share ell) and DMA'd in.
 * Scores are built transposed (keys on partitions, queries free). Softmax
   uses a per-query upper bound C_i instead of a row max (any per-query shift
   cancels in softmax); C_i rides the score matmul as one extra rank-1 row.
   The denominator comes from a ones-column appended to V in the attn@V
   matmul, and the division is applied after attn@V (cheaper).
"""

import math

import numpy as np

B, N, DIM, H, NF = 2, 1024, 512, 8, 8
HD = DIM // H
SCALE = HD ** -0.5
NCORES = 8
CHUNK = 128
NCHUNKS = N // CHUNK

_PROGRAM_CACHE = {}


def _softplus64(x):
    x = np.asarray(x, np.float64)
    return np.log1p(np.exp(-np.abs(x))) + np.maximum(x, 0.0) + 1e-12


def _split_excess_waits(nc, max_waits=1):
    """CoreV3 walrus allows only one sync-wait command on some instruction
    encodings; move excess waits onto preceding same-engine NoOps."""
    import concourse.mybir as mybir
    import bass_rust

    n_split = 0
    for bb in nc.main_func.blocks:
        new_list = []
        changed = False
        for ins in bb.instructions:
            si = ins.sync_info
            waits = list(si.on_wait) if (si and si.on_wait) else []
            if len(waits) > max_waits:
                changed = True
                extra, keep = waits[:-max_waits], waits[-max_waits:]
                for i in range(0, len(extra), max_waits):
                    chunk = extra[i : i + max_waits]
                    n_split += 1
                    new_list.append(
                        mybir.InstNoOp(
                            name=f"{ins.name}-ws{i}",
                            engine=ins.engine,
                            ins=[],
                            outs=[],
                            sync_info=bass_rust.SyncInfo(
                                on_wait=chunk, on_update=[]
                            ),
                        )
                    )
                si.on_wait = keep
            new_list.append(ins)
        if changed:
            bb.instructions = new_list
    return n_split


def _sin_regions(j0):
    """Column regions of uniform sign for the sin-part at key chunk j0.

    Returns a list of (c0, c1, negate) with every region inside one PSUM
    bank (512 fp32 columns)."""
    regions = []
    # queries left of the chunk: sign(x_i - x_j) = -1
    left = [(0, min(j0, 512), True), (512, j0, True)]
    # queries right of the chunk: +1
    r0 = j0 + CHUNK
    right = [(r0, min(512, N), False), (max(r0, 512), N, False)]
    for c0, c1, neg in left + right:
        if c1 > c0:
            regions.append((c0, c1, neg))
    return regions


def _build_program():
    if "nc" in _PROGRAM_CACHE:
        return _PROGRAM_CACHE["nc"]

    import concourse.bass as bass
    import concourse.mybir as mybir
    import concourse.tile as tile

    f32 = mybir.dt.float32
    Alu = mybir.AluOpType
    Act = mybir.ActivationFunctionType

    nc = bass.Bass(trn_type="TRN2")

    # ---- per-core DRAM I/O ------------------------------------------------
    xs_d = nc.dram_tensor("xs", [N, DIM], f32, kind="ExternalInput")
    ident_d = nc.dram_tensor("ident", [128, 128], f32, kind="ExternalInput")
    wqk_d = nc.dram_tensor("wqk", [DIM, 256], f32, kind="ExternalInput")
    wv_d = nc.dram_tensor("wv", [DIM, 128], f32, kind="ExternalInput")
    wproj_d = nc.dram_tensor("wproj", [128, DIM], f32, kind="ExternalInput")
    qb_d = nc.dram_tensor("qb", [2, HD, 1], f32, kind="ExternalInput")
    kb_d = nc.dram_tensor("kb", [2, HD, 1], f32, kind="ExternalInput")
    kext_d = nc.dram_tensor("kext", [2, 17, N], f32, kind="ExternalInput")
    qext_d = nc.dram_tensor("qext", [2, 17, N], f32, kind="ExternalInput")
    qc_d = nc.dram_tensor("qc", [2, 16, N], f32, kind="ExternalInput")
    qcn_d = nc.dram_tensor("qcn", [2, 16, N], f32, kind="ExternalInput")
    e_d = nc.dram_tensor("emat", [N, N], f32, kind="ExternalInput")
    ta_d = nc.dram_tensor("ta", [2, 128, 1], f32, kind="ExternalInput")
    tri_d = nc.dram_tensor("tri", [128, 128], f32, kind="ExternalInput")
    ones64_d = nc.dram_tensor("ones64", [1, 64], f32, kind="ExternalInput")
    yt_d = nc.dram_tensor("yt", [DIM, N], f32, kind="ExternalOutput")

    with tile.TileContext(nc) as tc:
        with (
            tc.tile_pool(name="persist", bufs=1) as pers,
            tc.tile_pool(name="work", bufs=3) as work,
            tc.tile_pool(name="dmw", bufs=2) as dmw,
            tc.tile_pool(name="yg", bufs=2) as ygp,
        ):
            # ---- persistent SBUF tiles + input DMA ------------------------
            def pt(shape, tag):
                return pers.tile(shape, f32, tag=tag, name=tag)

            ident_t = pt([128, 128], "ident")
            nc.sync.dma_start(ident_t[:], ident_d[:])
            tri_t = pt([128, 128], "tri")
            nc.sync.dma_start(tri_t[:], tri_d[:])
            ones64_t = pt([1, 64], "ones64")
            nc.sync.dma_start(ones64_t[:], ones64_d[:])

            xs_t = []
            for t in range(8):
                s = pt([128, DIM], f"xs{t}")
                nc.sync.dma_start(s[:], xs_d[t * 128 : (t + 1) * 128, :])
                xs_t.append(s)
            wqk_t = []
            for kc in range(4):
                s = pt([128, 256], f"wqk{kc}")
                nc.sync.dma_start(s[:], wqk_d[kc * 128 : (kc + 1) * 128, :])
                wqk_t.append(s)
            wv_t = []
            for kc in range(4):
                s = pt([128, 128], f"wv{kc}")
                nc.sync.dma_start(s[:], wv_d[kc * 128 : (kc + 1) * 128, :])
                wv_t.append(s)
            wproj_t = pt([128, DIM], "wproj")
            nc.sync.dma_start(wproj_t[:], wproj_d[:])

            e_t = []
            for k in range(NCHUNKS):
                s = pt([128, N], f"e{k}")
                nc.sync.dma_start(s[:], e_d[k * 128 : (k + 1) * 128, :])
                e_t.append(s)

            ta_t, qb_t, kb_t = [], [], []
            qc_t, qcn_t = [], []
            for h in range(2):
                s = pt([128, 1], f"ta{h}")
                nc.sync.dma_start(s[:], ta_d[h])
                ta_t.append(s)
                s = pt([HD, 1], f"qb{h}")
                nc.sync.dma_start(s[:], qb_d[h])
                qb_t.append(s)
                s = pt([HD, 1], f"kb{h}")
                nc.sync.dma_start(s[:], kb_d[h])
                kb_t.append(s)
                s = pt([16, N], f"qc{h}")
                nc.sync.dma_start(s[:], qc_d[h])
                qc_t.append(s)
                s = pt([16, N], f"qcn{h}")
                nc.sync.dma_start(s[:], qcn_d[h])
                qcn_t.append(s)

            kf_t = [pt([81, N], f"kf{h}") for h in range(2)]
            qa_t = [pt([81, N], f"qa{h}") for h in range(2)]
            feat_t = [pt([16, N], f"feat{h}") for h in range(2)]
            for h in range(2):
                nc.sync.dma_start(kf_t[h][64:81, :], kext_d[h])
                nc.sync.dma_start(qa_t[h][64:81, :], qext_d[h])
                nc.sync.dma_start(feat_t[h][:], kext_d[h, 0:16, :])

            vo_t = [[pt([128, 65], f"vo{h}_{t}") for t in range(8)] for h in range(2)]
            os_t = pt([128, N], "os")

            # ---- prolog: x^T, qk^T, v ------------------------------------
            with (
                tc.tile_pool(name="ppro", bufs=2, space="PSUM") as ppro,
                tc.tile_pool(name="pv", bufs=2, space="PSUM") as pvp,
            ):
                xT_t = [pt([128, N], f"xT{c}") for c in range(4)]
                for c in range(4):
                    p = ppro.tile([128, N], f32, tag="ppro")
                    for t in range(8):
                        nc.tensor.transpose(
                            p[:, t * 128 : (t + 1) * 128],
                            xs_t[t][:, c * 128 : (c + 1) * 128],
                            ident_t[:],
                        )
                    nc.vector.tensor_copy(xT_t[c][:], p[:])

                for h in range(2):
                    p = ppro.tile([128, N], f32, tag="ppro")
                    for kc in range(4):
                        for nh in range(2):
                            nc.tensor.matmul(
                                p[:, nh * 512 : (nh + 1) * 512],
                                lhsT=wqk_t[kc][:, h * 128 : (h + 1) * 128],
                                rhs=xT_t[kc][:, nh * 512 : (nh + 1) * 512],
                                start=(kc == 0),
                                stop=(kc == 3),
                            )
                    # q rows -> QA[0:64] with (q + qb) * scale
                    nc.vector.tensor_scalar(
                        qa_t[h][0:64, :],
                        p[0:64, :],
                        scalar1=qb_t[h][:],
                        scalar2=SCALE,
                        op0=Alu.add,
                        op1=Alu.mult,
                    )
                    # k rows -> KF[0:64] with + kb
                    nc.vector.tensor_scalar(
                        kf_t[h][0:64, :],
                        p[64:128, :],
                        scalar1=kb_t[h][:],
                        scalar2=None,
                        op0=Alu.add,
                    )

                for t in range(8):
                    p = pvp.tile([128, 128], f32, tag="pv")
                    for kc in range(4):
                        nc.tensor.matmul(
                            p[:],
                            lhsT=xT_t[kc][:, t * 128 : (t + 1) * 128],
                            rhs=wv_t[kc][:],
                            start=(kc == 0),
                            stop=(kc == 3),
                        )
                    for h in range(2):
                        nc.vector.tensor_copy(
                            vo_t[h][t][:, 0:64], p[:, h * 64 : (h + 1) * 64]
                        )
                        nc.vector.memset(vo_t[h][t][:, 64:65], 1.0)

            # ---- main attention loop -------------------------------------
            with (
                tc.tile_pool(name="pp", bufs=2, space="PSUM") as ppp,
                tc.tile_pool(name="pd", bufs=2, space="PSUM") as pdp,
                tc.tile_pool(name="po", bufs=1, space="PSUM") as pop,
            ):
                for h in range(2):
                    o = pop.tile([128, N], f32, tag="po")
                    for k in range(NCHUNKS):
                        j0 = k * 128
                        p = ppp.tile([128, N], f32, tag="pp")
                        regions = _sin_regions(j0)
                        for nh in range(2):
                            nc.tensor.matmul(
                                p[:, nh * 512 : (nh + 1) * 512],
                                lhsT=kf_t[h][:, j0 : j0 + 128],
                                rhs=qa_t[h][:, nh * 512 : (nh + 1) * 512],
                                start=True,
                                stop=False,
                                skip_group_check=True,
                            )
                        for ri, (c0, c1, neg) in enumerate(regions):
                            src = qcn_t[h] if neg else qc_t[h]
                            nc.tensor.matmul(
                                p[:, c0:c1],
                                lhsT=feat_t[h][:, j0 : j0 + 128],
                                rhs=src[:, c0:c1],
                                start=False,
                                stop=(ri == len(regions) - 1),
                                skip_group_check=True,
                            )
                        # diagonal 128x128 block: triangular sign fix
                        d = pdp.tile([128, 128], f32, tag="pd")
                        nc.tensor.matmul(
                            d[:],
                            lhsT=feat_t[h][:, j0 : j0 + 128],
                            rhs=qc_t[h][:, j0 : j0 + 128],
                            start=True,
                            stop=True,
                            skip_group_check=True,
                        )
                        dm = dmw.tile([128, 128], f32, tag="dm")
                        nc.vector.tensor_tensor(dm[:], d[:], tri_t[:], op=Alu.mult)
                        nc.vector.tensor_tensor(
                            p[:, j0 : j0 + 128], p[:, j0 : j0 + 128], dm[:],
                            op=Alu.add,
                        )
                        # scores = P + ta * E   (SBUF), then exp
                        ts = work.tile([128, N], f32, tag="ts")
                        nc.vector.scalar_tensor_tensor(
                            ts[:],
                            in0=e_t[k][:],
                            scalar=ta_t[h][:],
                            in1=p[:],
                            op0=Alu.mult,
                            op1=Alu.add,
                        )
                        xb = work.tile([128, N], f32, tag="xb")
                        nc.scalar.activation(xb[:], ts[:], Act.Exp)
                        for nh in range(2):
                            nc.tensor.matmul(
                                o[0:65, nh * 512 : (nh + 1) * 512],
                                lhsT=vo_t[h][k][:],
                                rhs=xb[:, nh * 512 : (nh + 1) * 512],
                                start=(k == 0),
                                stop=(k == NCHUNKS - 1),
                                skip_group_check=True,
                            )
                    # normalize: OS[h] = O[0:64] * (1/rowsum) broadcast
                    r = dmw.tile([1, N], f32, tag="rr")
                    nc.vector.reciprocal(r[:], o[64:65, :])
                    rb = ppp.tile([64, N], f32, tag="pp")
                    for nh in range(2):
                        nc.tensor.matmul(
                            rb[:, nh * 512 : (nh + 1) * 512],
                            lhsT=ones64_t[:],
                            rhs=r[:, nh * 512 : (nh + 1) * 512],
                            start=True,
                            stop=True,
                            skip_group_check=True,
                        )
                    nc.scalar.copy(os_t[h * 64 : (h + 1) * 64, :], o[0:64, :])
                    nc.vector.tensor_tensor(
                        os_t[h * 64 : (h + 1) * 64, :],
                        os_t[h * 64 : (h + 1) * 64, :],
                        rb[:],
                        op=Alu.mult,
                    )

                # ---- projection partial ----------------------------------
                for g in range(4):
                    p = ppp.tile([128, N], f32, tag="pp")
                    for nh in range(2):
                        nc.tensor.matmul(
                            p[:, nh * 512 : (nh + 1) * 512],
                            lhsT=wproj_t[:, g * 128 : (g + 1) * 128],
                            rhs=os_t[:, nh * 512 : (nh + 1) * 512],
                            start=True,
                            stop=True,
                            skip_group_check=True,
                        )
                    yg = ygp.tile([128, N], f32, tag="yg")
                    nc.scalar.copy(yg[:], p[:])
                    nc.sync.dma_start(yt_d[g * 128 : (g + 1) * 128, :], yg[:])

    _split_excess_waits(nc)
    _PROGRAM_CACHE["nc"] = nc
    return nc


def _prepare_in_maps(
    x_tokens, coords, qkv_w, qkv_b, proj_w, omega_raw, a, c,
    alpha_raw, ell_raw, bias_scale_raw,
):
    """Host-side preprocessing. Returns (in_maps, perms)."""
    x64 = np.asarray(x_tokens, np.float64)
    co64 = np.asarray(coords, np.float64)
    w64 = np.asarray(qkv_w, np.float64)
    wb64 = np.asarray(qkv_b, np.float64)

    alpha = _softplus64(alpha_raw)            # (H,)
    ell = _softplus64(ell_raw)                # (H,)
    om = _softplus64(omega_raw)               # (H, F)
    t = np.tanh(np.asarray(bias_scale_raw, np.float64))  # (H,)
    a2 = t[:, None] * np.asarray(a, np.float64)          # (H, F)
    c2 = t[:, None] * np.asarray(c, np.float64)
    ta = t * alpha                                        # (H,)

    assert np.allclose(ell, ell[0]), "per-head ell not supported"

    ident = np.eye(128, dtype=np.float32)
    io, jo = np.meshgrid(np.arange(128), np.arange(128), indexing="ij")
    tri = np.sign(jo - io).astype(np.float32)  # TRI[p, c] = sign(c - p)
    ones64 = np.ones((1, 64), np.float32)

    perms, in_maps = [], []
    for b in range(B):
        perm = np.argsort(co64[b], kind="stable")
        perms.append(perm)
        cs = co64[b][perm]                      # sorted coords
        xs = x64[b][perm]                       # (N, DIM)
        d2 = (cs[:, None] - cs[None, :]) ** 2
        emat = np.exp(-d2 / (ell[0] ** 2)).astype(np.float32)

        # all-head q/k in f64 for the per-query shift bound
        qk = xs @ w64[:, : 2 * DIM] + wb64[: 2 * DIM]
        for pair in range(4):
            heads = (2 * pair, 2 * pair + 1)
            wqk_cols, wv_cols = [], []
            qb_rows, kb_rows = [], []
            kext, qext, qcm, qcnm = [], [], [], []
            ta_rows = []
            for h in heads:
                sl_q = slice(h * HD, (h + 1) * HD)
                sl_k = slice(DIM + h * HD, DIM + (h + 1) * HD)
                sl_v = slice(2 * DIM + h * HD, 2 * DIM + (h + 1) * HD)
                wqk_cols.append(np.asarray(qkv_w)[:, sl_q])
                wqk_cols.append(np.asarray(qkv_w)[:, sl_k])
                wv_cols.append(np.asarray(qkv_w)[:, sl_v])
                qb_rows.append(np.asarray(qkv_b)[sl_q])
                kb_rows.append(np.asarray(qkv_b)[sl_k])

                C = np.cos(om[h][:, None] * cs[None, :])   # (F, N)
                S = np.sin(om[h][:, None] * cs[None, :])
                kext.append(
                    np.concatenate(
                        [C, S, np.ones((1, N))], axis=0
                    )
                )
                qn = np.linalg.norm(qk[:, sl_q], axis=1)   # (N,)
                kmax = np.linalg.norm(qk[:, sl_k], axis=1).max()
                bb = abs(ta[h]) + np.abs(a2[h]).sum() + np.abs(c2[h]).sum()
                ci = SCALE * qn * kmax + bb + 1.0          # (N,)
                qext.append(
                    np.concatenate(
                        [a2[h][:, None] * C, a2[h][:, None] * S, -ci[None, :]],
                        axis=0,
                    )
                )
                qcm.append(
                    np.concatenate(
                        [c2[h][:, None] * S, -c2[h][:, None] * C], axis=0
                    )
                )
                qcnm.append(-qcm[-1])
                ta_rows.append(np.full((128, 1), ta[h]))

            in_maps.append(
                {
                    "xs": np.ascontiguousarray(xs, dtype=np.float32),
                    "ident": ident,
                    "wqk": np.ascontiguousarray(
                        np.concatenate(wqk_cols, axis=1), dtype=np.float32
                    ),
                    "wv": np.ascontiguousarray(
                        np.concatenate(wv_cols, axis=1), dtype=np.float32
                    ),
                    "wproj": np.ascontiguousarray(
                        np.asarray(proj_w)[
                            heads[0] * HD : (heads[1] + 1) * HD, :
                        ],
                        dtype=np.float32,
                    ),
                    "qb": np.stack(qb_rows).astype(np.float32)[:, :, None],
                    "kb": np.stack(kb_rows).astype(np.float32)[:, :, None],
                    "kext": np.stack(kext).astype(np.float32),
                    "qext": np.stack(qext).astype(np.float32),
                    "qc": np.stack(qcm).astype(np.float32),
                    "qcn": np.stack(qcnm).astype(np.float32),
                    "emat": emat,
                    "ta": np.stack(ta_rows).astype(np.float32),
                    "tri": tri,
                    "ones64": ones64,
                }
            )
    return in_maps, perms


def kernel(
    x_tokens, coords, qkv_w, qkv_b, proj_w, proj_b,
    omega_raw, a, c, alpha_raw, ell_raw, bias_scale_raw,
):
    from concourse.bass_utils import run_bass_kernel_spmd

    nc = _build_program()
    in_maps, perms = _prepare_in_maps(
        x_tokens, coords, qkv_w, qkv_b, proj_w, omega_raw, a, c,
        alpha_raw, ell_raw, bias_scale_raw,
    )
    res = run_bass_kernel_spmd(nc, in_maps, core_ids=list(range(NCORES)))

    # v-bias contributes a constant row (attention weights sum to 1)
    vb = np.asarray(qkv_b, np.float64)[2 * DIM :]
    const_row = vb @ np.asarray(proj_w, np.float64) + np.asarray(
        proj_b, np.float64
    )

    out = np.empty((B, N, DIM), np.float32)
    for b in range(B):
        acc = np.zeros((N, DIM), np.float64)
        for pair in range(4):
            acc += res.results[4 * b + pair]["yt"].T.astype(np.float64)
        acc += const_row[None, :]
        y = np.empty((N, DIM), np.float64)
        y[perms[b]] = acc
        out[b] = y.astype(np.float32)
    return out
